# revision 48
# baseline (speedup 1.0000x reference)
"""Local (7x7 windowed) attention Trainium2 kernel — fp16 tensor-engine path.

Problem: B=1, N=4096 (T=4, H=W=32), C=384, 8 heads x hd=48, window 7x7
zero-padded (reference semantics: padded keys score exactly 0 -> weight
exp(0), value 0).

Sharding: data-parallel over positions. 8 cores; core c owns t-slice
c//2, query rows [16*(c%2), 16*(c%2)+16) (512 queries). Each core
recomputes k/v for a 3-row halo (24 rows = 768 halo positions,
zero-padded outside the image, matching the reference's zero padding).

v3 design: all matmul operands fp16 (1 cyc/row on the PE array vs 4 for
fp32). Per (head-pair pr, head e) the 6 key j-tiles (4 halo rows each)
pack into one [128, 1280] PSUM tile at offsets chosen so no matmul
output crosses a 2KB PSUM bank (jt2@0, jt4@320 | jt3@512, jt0@832,
jt5@960 | jt1@1024), then ONE exp ACT covers the whole tile and one DVE
multiply applies the 0/1 window mask (padded-key exp(0)=1 terms enter
the denominator via a noob-count init matmul; ones column 0 of the
augmented V produces the denominator as row 0/64 of the AV output).
All four pairs' AV outputs live in one [128, 2048] PSUM tile so the
denominator extraction, reciprocal, broadcast-to-fp16 and normalize
multiply each run once. Tiles are split per-pr / per-pt so whole-tile
dependencies pipeline (subtile dep tracking is disabled: it misses
write->read deps on strided writes here, giving nondeterministic
results).
"""

import os

# the tile scheduler's subtile dependency tracking misses write->read deps
# on this kernel's strided vaug writes (observed as nondeterministic output);
# coarse whole-tile deps are correct
os.environ["BY_DEFAULT_DISABLE_SUBTILE_DEPS"] = "1"

import numpy as np

import concourse.bacc as bacc
import concourse.mybir as mybir
import concourse.tile as tile
from concourse.bass_utils import run_bass_kernel_spmd

F = mybir.dt.float32
H = mybir.dt.float16

NH = 8
HD = 48
WIN = 7
HALF = 3
T, HH, WW = 4, 32, 32
C = 384
NPOS = T * HH * WW
SCALE = HD ** -0.5

# per j-tile (4 halo key rows each): (i_lo, span, mask_col_offset)
SPANS = [
    (0, 128, 192),
    (0, 256, 64),
    (64, 320, 0),
    (192, 320, 0),
    (320, 192, 0),
    (448, 64, 0),
]
# packed offsets of each j-tile's span inside the [128, 1280] score tile
JTOFF = {2: 0, 4: 320, 3: 512, 0: 832, 5: 960, 1: 1024}

_CACHE = {}
LAST_RESULT = None


def _build_nc():
    if "nc" in _CACHE:
        return _CACHE["nc"]
    nc = bacc.Bacc("TRN2", target_bir_lowering=False)

    d_xT = [nc.dram_tensor(f"xT{k}", [128, 768], H, kind="ExternalInput")
            for k in range(3)]
    d_wqk = [nc.dram_tensor(f"wqk{k}", [128, 8, 128], H, kind="ExternalInput")
             for k in range(3)]
    d_wv = nc.dram_tensor("wv", [128, 3, 384], H, kind="ExternalInput")
    d_wp = nc.dram_tensor("wp", [128, 4, 384], H, kind="ExternalInput")
    d_mask = nc.dram_tensor("maskm", [128, 1280], H, kind="ExternalInput")
    d_noob = nc.dram_tensor("noobh", [1, 512], H, kind="ExternalInput")
    d_sel = nc.dram_tensor("sel2", [1, 128], H, kind="ExternalInput")
    d_out = nc.dram_tensor("out", [512, 384], H, kind="ExternalOutput")

    EXP = mybir.ActivationFunctionType.Exp

    with tile.TileContext(nc) as tc:
        with tc.tile_pool(name="singles", bufs=1) as S:
            xT = [S.tile([128, 768], H, name=f"xT{k}") for k in range(3)]
            wqk = [S.tile([128, 8, 128], H, name=f"wqk{k}") for k in range(3)]
            wv = S.tile([128, 3, 384], H)
            wp = S.tile([128, 4, 384], H)
            maskm = S.tile([128, 1280], H)
            noobh = S.tile([1, 512], H)
            sel2 = S.tile([1, 128], H)
            # per-pr packed [q(512) | kA(512) | kB(256)] fp16
            qk = [S.tile([128, 1280], H, name=f"qk{pr}") for pr in range(4)]
            # per-pt augmented V: col 0 = ones (denominator), 1:49 = v
            vaug = [S.tile([128, 8, 64], H, name=f"vaug{pt}") for pt in range(6)]
            nhat = [S.tile([128, 512], H, name=f"nhat{pr}") for pr in range(4)]

            # input DMAs ordered by first use, dispatch split across the
            # two DMA-capable engines (SP + Activation) to halve the serial
            # ~0.6us-per-dispatch warmup
            for k in range(3):
                nc.sync.dma_start(out=wqk[k][:], in_=d_wqk[k][:])
                nc.sync.dma_start(out=xT[k][:], in_=d_xT[k][:])
            for sb, dr in [
                (wv, d_wv), (maskm, d_mask), (noobh, d_noob),
                (sel2, d_sel), (wp, d_wp),
            ]:
                nc.scalar.dma_start(out=sb[:], in_=dr[:])
            for pt in range(6):
                nc.gpsimd.memset(vaug[pt][:, :, 0:1], 1.0)
                nc.gpsimd.memset(vaug[pt][:, :, 49:64], 0.0)

            # ---- phase 1: [q|k] per head-pair, v natural -------------
            # emit pr 0/1 first so phase-2 score matmuls can start while
            # v and the remaining pairs are still being produced
            with tc.tile_pool(name="psA", bufs=2, space="PSUM") as psA:
                def emit_qk(pr):
                    QK = psA.tile([128, 1280], F, tag="qk", name=f"QK{pr}")
                    for k in range(3):
                        st, sp_ = (k == 0), (k == 2)
                        nc.tensor.matmul(QK[:, 0:512], wqk[k][:, 2 * pr, :],
                                         xT[k][:, 96:608], start=st, stop=sp_)
                        nc.tensor.matmul(QK[:, 512:1024],
                                         wqk[k][:, 2 * pr + 1, :],
                                         xT[k][:, 0:512], start=st, stop=sp_)
                        nc.tensor.matmul(QK[:, 1024:1280],
                                         wqk[k][:, 2 * pr + 1, :],
                                         xT[k][:, 512:768], start=st, stop=sp_)
                    # split across engines to halve the copy latency
                    nc.vector.tensor_copy(qk[pr][:, 0:512], QK[:, 0:512])
                    nc.scalar.copy(qk[pr][:, 512:1280], QK[:, 512:1280])

                emit_qk(0)
                emit_qk(1)
                for pt in range(6):
                    V = psA.tile([128, 384], F, tag="V")
                    for k in range(3):
                        nc.tensor.matmul(V[:], xT[k][:, 128 * pt:128 * pt + 128],
                                         wv[:, k, :], start=(k == 0), stop=(k == 2))
                    nc.scalar.copy(
                        vaug[pt][:, :, 1:49],
                        V[:].rearrange("p (h d) -> p h d", h=8))
                emit_qk(2)
                emit_qk(3)

            # ---- phases 2-4, software-pipelined ----------------------
            # Engines execute their instruction queues in FIFO order, so
            # emission order IS schedule: interleave so no tensor-engine
            # instruction waits on a chain younger than ~2 iterations.
            with tc.tile_pool(name="psS", bufs=2, space="PSUM") as psS, \
                 tc.tile_pool(name="psO", bufs=1, space="PSUM") as psO, \
                 tc.tile_pool(name="sbe", bufs=3) as sbe, \
                 tc.tile_pool(name="sbn", bufs=2) as sbn:
                O2s = [None] * 4
                eTs = [None] * 8
                rechs = [None] * 4
                Bcs = [None] * 4

                def emit_S(i):
                    pr, e = divmod(i, 2)
                    sS = psS.tile([128, 1280], F, tag="sS", name=f"sS{i}")
                    for jt in range(6):
                        ilo, spn, _ = SPANS[jt]
                        off = JTOFF[jt]
                        nc.tensor.matmul(
                            sS[:, off:off + spn],
                            qk[pr][64 * e:64 * e + 64,
                                   512 + 128 * jt:512 + 128 * (jt + 1)],
                            qk[pr][64 * e:64 * e + 64, ilo:ilo + spn],
                            start=True, stop=True)
                    eTu = sbe.tile([128, 1280], H, tag="eTu", name=f"eTu{i}")
                    nc.scalar.activation(eTu[:], sS[:], EXP, scale=SCALE)
                    eT = sbe.tile([128, 1280], H, tag="eT", name=f"eT{i}")
                    nc.vector.tensor_mul(eT[:], eTu[:], maskm[:])
                    eTs[i] = eT

                def emit_AV(i):
                    pr, e = divmod(i, 2)
                    h = i
                    if e == 0:
                        O2s[pr] = psO.tile([128, 512], F, tag="O", bufs=2,
                                           name=f"O2_{pr}")
                        # one init for both heads: noob into den rows 0 and 64
                        nc.tensor.matmul(O2s[pr][:, :], sel2[:], noobh[:],
                                         start=True, stop=False,
                                         skip_group_check=True)
                    O2 = O2s[pr]
                    for jt in range(6):
                        ilo, spn, _ = SPANS[jt]
                        off = JTOFF[jt]
                        nc.tensor.matmul(
                            O2[64 * e:64 * e + 64, ilo:ilo + spn],
                            vaug[jt][:, h, :],
                            eTs[i][:, off:off + spn],
                            start=False, stop=(jt == 5),
                            skip_group_check=True)

                def emit_chain(pr):
                    O2 = O2s[pr]
                    denB = sbn.tile([1, 512], F, tag="denB")
                    nc.scalar.copy(denB[:], O2[64:65, :])
                    recA = sbn.tile([1, 512], F, tag="recA")
                    recB = sbn.tile([1, 512], F, tag="recB")
                    # custom-DVE ops drop PSUM partition offsets; row 0 is ok
                    nc.vector.reciprocal_approx_fast(out=recA[:],
                                                     in_=O2[0:1, :])
                    nc.vector.reciprocal_approx_fast(out=recB[:], in_=denB[:])
                    rechA = sbn.tile([1, 512], H, tag="rechA")
                    rechB = sbn.tile([1, 512], H, tag="rechB")
                    nc.scalar.copy(rechA[:], recA[:])
                    nc.scalar.copy(rechB[:], recB[:])
                    rechs[pr] = (rechA, rechB)

                def emit_Bc(pr):
                    rechA, rechB = rechs[pr]
                    # broadcast 1/den across partitions on the idle gpsimd
                    # (its ucode mishandles output partition offsets, so two
                    # separate partition-0 tiles)
                    BcE = sbn.tile([64, 512], H, tag="BcE")
                    BcO = sbn.tile([64, 512], H, tag="BcO")
                    nc.gpsimd.partition_broadcast(BcE[:], rechA[:])
                    nc.gpsimd.partition_broadcast(BcO[:], rechB[:])
                    Bcs[pr] = (BcE, BcO)

                def emit_nhat(pr):
                    BcE, BcO = Bcs[pr]
                    nc.vector.tensor_mul(nhat[pr][0:64, :],
                                         O2s[pr][0:64, :], BcE[:])
                    nc.vector.tensor_mul(nhat[pr][64:128, :],
                                         O2s[pr][64:128, :], BcO[:])

                emit_S(0)
                emit_S(1)
                emit_AV(0)
                emit_S(2)
                emit_AV(1)
                emit_chain(0)
                emit_S(3)
                emit_Bc(0)
                emit_nhat(0)
                emit_AV(2)
                emit_S(4)
                emit_AV(3)
                emit_chain(1)
                emit_S(5)
                emit_Bc(1)
                emit_nhat(1)
                emit_AV(4)
                emit_S(6)
                emit_AV(5)
                emit_chain(2)
                emit_S(7)
                emit_Bc(2)
                emit_nhat(2)
                emit_AV(6)
                emit_AV(7)
                emit_chain(3)
                emit_Bc(3)
                emit_nhat(3)

            # ---- phase 5: projection + bias --------------------------
            with tc.tile_pool(name="psP", bufs=2, space="PSUM") as psP, \
                 tc.tile_pool(name="sbo", bufs=2) as sbo:
                for it in range(4):
                    # bias comes via wp row 0 (nhat row 0 = den*rec = 1.0)
                    P = psP.tile([128, 384], F, tag="P")
                    for pr in range(4):
                        nc.tensor.matmul(
                            P[:], nhat[pr][:, 128 * it:128 * (it + 1)],
                            wp[:, pr, :], start=(pr == 0), stop=(pr == 3))
                    ot = sbo.tile([128, 384], H, tag="ot")
                    nc.vector.tensor_copy(ot[:], P[:])
                    nc.sync.dma_start(out=d_out[128 * it:128 * (it + 1), :],
                                      in_=ot[:])

    nc.compile()
    _CACHE["nc"] = nc
    return nc


def _host_consts(w_qkv, w_proj, b_proj):
    wqk = np.zeros((3, 128, 8, 128), np.float16)
    for k in range(3):
        rows = slice(k * 128, (k + 1) * 128)
        for pr in range(4):
            for s in range(2):  # 0 = q block, 1 = k block
                off = 384 * s
                wqk[k, :, 2 * pr + s, 0:48] = \
                    w_qkv[rows, off + 48 * (2 * pr):off + 48 * (2 * pr) + 48]
                wqk[k, :, 2 * pr + s, 64:112] = \
                    w_qkv[rows, off + 48 * (2 * pr + 1):off + 48 * (2 * pr + 1) + 48]
    wv = np.ascontiguousarray(np.transpose(
        w_qkv[:, 768:1152].reshape(3, 128, 384), (1, 0, 2))).astype(np.float16)
    wp = np.zeros((128, 4, 384), np.float16)
    for pr in range(4):  # +1: row 0 / 64 of nhat is the denominator row
        wp[1:49, pr, :] = w_proj[96 * pr:96 * pr + 48, :]
        wp[65:113, pr, :] = w_proj[96 * pr + 48:96 * pr + 96, :]
    # nhat row 0 is den*recip(den) == 1.0, so wp row 0 carries the bias
    wp[0, 0, :] = b_proj

    base = np.zeros((128, 320), np.float16)  # 1 = key allowed for query
    for r in range(4):
        for q in range(10):
            if r <= q <= r + 6:
                xj, xi = np.meshgrid(np.arange(32), np.arange(32), indexing="ij")
                base[32 * r:32 * r + 32, 32 * q:32 * q + 32] = \
                    (np.abs(xj - xi) <= 3).astype(np.float16)
    maskm = np.zeros((128, 1280), np.float16)
    for jt in range(6):
        ilo, spn, mo = SPANS[jt]
        maskm[:, JTOFF[jt]:JTOFF[jt] + spn] = base[:, mo:mo + spn]

    noobh = np.zeros((1, 512), np.float16)
    for qy in range(16):
        for qx in range(32):
            noobh[0, 32 * qy + qx] = 7.0 * (max(0, 3 - qx) + max(0, qx - 28))
    sel2 = np.zeros((1, 128), np.float16)
    sel2[0, 0] = 1.0
    sel2[0, 64] = 1.0
    d = dict(wv=wv, wp=wp, maskm=maskm, noobh=noobh, sel2=sel2)
    for k in range(3):
        d[f"wqk{k}"] = np.ascontiguousarray(wqk[k])
    return d


def kernel(x, w_qkv, w_proj, b_proj, H=32, W=32):
    global LAST_RESULT
    x = np.asarray(x, np.float32)
    w_qkv = np.asarray(w_qkv, np.float32)
    w_proj = np.asarray(w_proj, np.float32)
    b_proj = np.asarray(b_proj, np.float32)
    assert x.shape == (1, NPOS, C) and int(H) == 32 and int(W) == 32

    nc = _build_nc()
    consts = _host_consts(w_qkv, w_proj, b_proj)

    x4 = x[0].reshape(T, HH, WW, C)
    in_maps = []
    for c in range(8):
        t, ry0 = c // 2, 16 * (c % 2)
        xh = np.zeros((24, WW, C), np.float32)
        lo, hi = ry0 - 3, ry0 + 21
        slo, shi = max(lo, 0), min(hi, HH)
        xh[slo - lo:shi - lo] = x4[t, slo:shi]
        xTf = np.ascontiguousarray(
            xh.reshape(768, C).T.reshape(3, 128, 768)).astype(np.float16)
        m = {f"xT{k}": np.ascontiguousarray(xTf[k]) for k in range(3)}
        in_maps.append({**m, **consts})

    trace = bool(int(os.environ.get("TRACE", "0")))
    res = run_bass_kernel_spmd(nc, in_maps, core_ids=list(range(8)),
                               trace=trace)
    LAST_RESULT = res
    out = np.concatenate(
        [res.results[c]["out"].astype(np.float32) for c in range(8)], axis=0)
    return out.reshape(1, NPOS, C)


# revision 54
# speedup vs baseline: 1.0628x; 1.0628x over previous
"""Local (7x7 windowed) attention Trainium2 kernel — fp16 tensor-engine path.

Problem: B=1, N=4096 (T=4, H=W=32), C=384, 8 heads x hd=48, window 7x7
zero-padded (reference semantics: padded keys score exactly 0 -> weight
exp(0), value 0).

Sharding: data-parallel over positions. 8 cores; core c owns t-slice
c//2, query rows [16*(c%2), 16*(c%2)+16) (512 queries). Each core
recomputes k/v for a 3-row halo (24 rows = 768 halo positions,
zero-padded outside the image, matching the reference's zero padding).

v3 design: all matmul operands fp16 (1 cyc/row on the PE array vs 4 for
fp32). Per (head-pair pr, head e) the 6 key j-tiles (4 halo rows each)
pack into one [128, 1280] PSUM tile at offsets chosen so no matmul
output crosses a 2KB PSUM bank (jt2@0, jt4@320 | jt3@512, jt0@832,
jt5@960 | jt1@1024), then ONE exp ACT covers the whole tile and one DVE
multiply applies the 0/1 window mask (padded-key exp(0)=1 terms enter
the denominator via a noob-count init matmul; ones column 0 of the
augmented V produces the denominator as row 0/64 of the AV output).
All four pairs' AV outputs live in one [128, 2048] PSUM tile so the
denominator extraction, reciprocal, broadcast-to-fp16 and normalize
multiply each run once. Tiles are split per-pr / per-pt so whole-tile
dependencies pipeline (subtile dep tracking is disabled: it misses
write->read deps on strided writes here, giving nondeterministic
results).
"""

import os

# the tile scheduler's subtile dependency tracking misses write->read deps
# on this kernel's strided vaug writes (observed as nondeterministic output);
# coarse whole-tile deps are correct
os.environ["BY_DEFAULT_DISABLE_SUBTILE_DEPS"] = "1"

import numpy as np

import concourse.bacc as bacc
import concourse.mybir as mybir
import concourse.tile as tile
from concourse.bass_utils import run_bass_kernel_spmd

F = mybir.dt.float32
H = mybir.dt.float16

NH = 8
HD = 48
WIN = 7
HALF = 3
T, HH, WW = 4, 32, 32
C = 384
NPOS = T * HH * WW
SCALE = HD ** -0.5

# per j-tile (4 halo key rows each): (i_lo, span, mask_col_offset)
SPANS = [
    (0, 128, 192),
    (0, 256, 64),
    (64, 320, 0),
    (192, 320, 0),
    (320, 192, 0),
    (448, 64, 0),
]
# packed offsets of each j-tile's span inside the [128, 1280] score tile
JTOFF = {2: 0, 4: 320, 3: 512, 0: 832, 5: 960, 1: 1024}

_CACHE = {}
LAST_RESULT = None


def _build_nc():
    if "nc" in _CACHE:
        return _CACHE["nc"]
    nc = bacc.Bacc("TRN2", target_bir_lowering=False)

    d_xT = [nc.dram_tensor(f"xT{k}", [128, 768], H, kind="ExternalInput")
            for k in range(3)]
    d_wqk = [nc.dram_tensor(f"wqk{k}", [128, 8, 128], H, kind="ExternalInput")
             for k in range(3)]
    d_wv = nc.dram_tensor("wv", [128, 3, 384], H, kind="ExternalInput")
    d_wp = nc.dram_tensor("wp", [128, 4, 384], H, kind="ExternalInput")
    d_mask = nc.dram_tensor("maskm", [128, 1280], H, kind="ExternalInput")
    d_noob = nc.dram_tensor("noobh", [1, 512], H, kind="ExternalInput")
    d_sel = nc.dram_tensor("sel2", [1, 128], H, kind="ExternalInput")
    d_bselE = nc.dram_tensor("bselE", [1, 128], H, kind="ExternalInput")
    d_bselO = nc.dram_tensor("bselO", [1, 128], H, kind="ExternalInput")
    d_out = nc.dram_tensor("out", [512, 384], H, kind="ExternalOutput")

    EXP = mybir.ActivationFunctionType.Exp

    with tile.TileContext(nc) as tc:
        with tc.tile_pool(name="singles", bufs=1) as S:
            xT = [S.tile([128, 768], H, name=f"xT{k}") for k in range(3)]
            wqk = [S.tile([128, 8, 128], H, name=f"wqk{k}") for k in range(3)]
            wv = S.tile([128, 3, 384], H)
            wp = S.tile([128, 4, 384], H)
            maskm = S.tile([128, 1280], H)
            noobh = S.tile([1, 512], H)
            sel2 = S.tile([1, 128], H)
            bselE = S.tile([1, 128], H)
            bselO = S.tile([1, 128], H)
            # per-pr packed [q(512) | kA(512) | kB(256)] fp16
            qk = [S.tile([128, 1280], H, name=f"qk{pr}") for pr in range(4)]
            # per-pt augmented V: col 0 = ones (denominator), 1:49 = v
            vaug = [S.tile([128, 8, 64], H, name=f"vaug{pt}") for pt in range(6)]
            nhat = [S.tile([128, 512], H, name=f"nhat{pr}") for pr in range(4)]

            # input DMAs, ordered by first use
            for k in range(3):
                nc.sync.dma_start(out=wqk[k][:], in_=d_wqk[k][:])
                nc.sync.dma_start(out=xT[k][:], in_=d_xT[k][:])
            for sb, dr in [
                (wv, d_wv), (maskm, d_mask), (noobh, d_noob),
                (sel2, d_sel), (bselE, d_bselE), (bselO, d_bselO),
                (wp, d_wp),
            ]:
                nc.sync.dma_start(out=sb[:], in_=dr[:])
            for pt in range(6):
                nc.gpsimd.memset(vaug[pt][:, :, 0:1], 1.0)
                nc.gpsimd.memset(vaug[pt][:, :, 49:64], 0.0)

            # ---- phase 1: [q|k] per head-pair, v natural -------------
            # emit pr 0/1 first so phase-2 score matmuls can start while
            # v and the remaining pairs are still being produced
            with tc.tile_pool(name="psA", bufs=2, space="PSUM") as psA:
                def emit_qk(pr):
                    QK = psA.tile([128, 1280], F, tag="qk", name=f"QK{pr}")
                    for k in range(3):
                        st, sp_ = (k == 0), (k == 2)
                        nc.tensor.matmul(QK[:, 0:512], wqk[k][:, 2 * pr, :],
                                         xT[k][:, 96:608], start=st, stop=sp_)
                        nc.tensor.matmul(QK[:, 512:1024],
                                         wqk[k][:, 2 * pr + 1, :],
                                         xT[k][:, 0:512], start=st, stop=sp_)
                        nc.tensor.matmul(QK[:, 1024:1280],
                                         wqk[k][:, 2 * pr + 1, :],
                                         xT[k][:, 512:768], start=st, stop=sp_)
                    # split across engines to halve the copy latency
                    nc.vector.tensor_copy(qk[pr][:, 0:512], QK[:, 0:512])
                    nc.scalar.copy(qk[pr][:, 512:1280], QK[:, 512:1280])

                emit_qk(0)
                emit_qk(1)
                for pt in range(6):
                    V = psA.tile([128, 384], F, tag="V")
                    for k in range(3):
                        nc.tensor.matmul(V[:], xT[k][:, 128 * pt:128 * pt + 128],
                                         wv[:, k, :], start=(k == 0), stop=(k == 2))
                    nc.scalar.copy(
                        vaug[pt][:, :, 1:49],
                        V[:].rearrange("p (h d) -> p h d", h=8))
                emit_qk(2)
                emit_qk(3)

            # ---- phases 2-4, software-pipelined ----------------------
            # Engines execute their instruction queues in FIFO order, so
            # emission order IS schedule: interleave so no tensor-engine
            # instruction waits on a chain younger than ~2 iterations.
            with tc.tile_pool(name="psS", bufs=2, space="PSUM") as psS, \
                 tc.tile_pool(name="psO", bufs=1, space="PSUM") as psO, \
                 tc.tile_pool(name="sbe", bufs=3) as sbe, \
                 tc.tile_pool(name="sbn", bufs=2) as sbn:
                O2s = [None] * 4
                eTs = [None] * 8
                rechs = [None] * 4
                Bcs = [None] * 4

                def emit_S(i):
                    pr, e = divmod(i, 2)
                    sS = psS.tile([128, 1280], F, tag="sS", name=f"sS{i}")
                    for jt in range(6):
                        ilo, spn, _ = SPANS[jt]
                        off = JTOFF[jt]
                        nc.tensor.matmul(
                            sS[:, off:off + spn],
                            qk[pr][64 * e:64 * e + 64,
                                   512 + 128 * jt:512 + 128 * (jt + 1)],
                            qk[pr][64 * e:64 * e + 64, ilo:ilo + spn],
                            start=True, stop=True)
                    # split exp+mask at the 1024 boundary: the first chunk
                    # covers j-tiles 2,4,3,0,5 so their AV matmuls can start
                    # while jt1's exp/mask still runs
                    eTu = sbe.tile([128, 1280], H, tag="eTu", name=f"eTu{i}")
                    eT = sbe.tile([128, 1280], H, tag="eT", name=f"eT{i}")
                    nc.scalar.activation(eTu[:, 0:1024], sS[:, 0:1024], EXP,
                                         scale=SCALE)
                    nc.vector.tensor_mul(eT[:, 0:1024], eTu[:, 0:1024],
                                         maskm[:, 0:1024])
                    nc.scalar.activation(eTu[:, 1024:1280], sS[:, 1024:1280],
                                         EXP, scale=SCALE)
                    nc.vector.tensor_mul(eT[:, 1024:1280], eTu[:, 1024:1280],
                                         maskm[:, 1024:1280])
                    eTs[i] = eT

                def emit_AV(i):
                    pr, e = divmod(i, 2)
                    h = i
                    if e == 0:
                        O2s[pr] = psO.tile([128, 512], F, tag="O", bufs=2,
                                           name=f"O2_{pr}")
                        # one init for both heads: noob into den rows 0 and 64
                        nc.tensor.matmul(O2s[pr][:, :], sel2[:], noobh[:],
                                         start=True, stop=False,
                                         skip_group_check=True)
                    O2 = O2s[pr]
                    for jt in (2, 4, 3, 0, 5, 1):  # jt1 last: its eT chunk lands last
                        ilo, spn, _ = SPANS[jt]
                        off = JTOFF[jt]
                        nc.tensor.matmul(
                            O2[64 * e:64 * e + 64, ilo:ilo + spn],
                            vaug[jt][:, h, :],
                            eTs[i][:, off:off + spn],
                            start=False, stop=(jt == 1),
                            skip_group_check=True)

                def emit_chain(pr):
                    O2 = O2s[pr]
                    denB = sbn.tile([1, 512], F, tag="denB")
                    nc.scalar.copy(denB[:], O2[64:65, :])
                    recA = sbn.tile([1, 512], F, tag="recA")
                    recB = sbn.tile([1, 512], F, tag="recB")
                    # custom-DVE ops drop PSUM partition offsets; row 0 is ok
                    nc.vector.reciprocal_approx_fast(out=recA[:],
                                                     in_=O2[0:1, :])
                    nc.vector.reciprocal_approx_fast(out=recB[:], in_=denB[:])
                    rechA = sbn.tile([1, 512], H, tag="rechA")
                    rechB = sbn.tile([1, 512], H, tag="rechB")
                    nc.scalar.copy(rechA[:], recA[:])
                    nc.scalar.copy(rechB[:], recB[:])
                    rechs[pr] = (rechA, rechB)

                def emit_Bc(pr):
                    rechA, rechB = rechs[pr]
                    # reuse a psS ring slot (free once that pair's exp read it)
                    Bc = psS.tile([128, 512], F, tag="sS", name=f"Bc{pr}")
                    nc.tensor.matmul(Bc[:], bselE[:], rechA[:],
                                     start=True, stop=False)
                    nc.tensor.matmul(Bc[:], bselO[:], rechB[:],
                                     start=False, stop=True)
                    Bcs[pr] = Bc

                def emit_nhat(pr):
                    BcS = sbn.tile([128, 512], H, tag="BcS")
                    nc.scalar.copy(BcS[:], Bcs[pr][:])
                    nc.vector.tensor_mul(nhat[pr][:], O2s[pr][:], BcS[:])

                emit_S(0)
                emit_S(1)
                emit_AV(0)
                emit_S(2)
                emit_AV(1)
                emit_chain(0)
                emit_S(3)
                emit_Bc(0)
                emit_nhat(0)
                emit_AV(2)
                emit_S(4)
                emit_AV(3)
                emit_chain(1)
                emit_S(5)
                emit_Bc(1)
                emit_nhat(1)
                emit_AV(4)
                emit_S(6)
                emit_AV(5)
                emit_chain(2)
                emit_S(7)
                emit_Bc(2)
                emit_nhat(2)
                emit_AV(6)
                emit_AV(7)
                emit_chain(3)
                emit_Bc(3)
                emit_nhat(3)

            # ---- phase 5: projection + bias --------------------------
            with tc.tile_pool(name="psP", bufs=2, space="PSUM") as psP, \
                 tc.tile_pool(name="sbo", bufs=2) as sbo:
                for it in range(4):
                    # bias comes via wp row 0 (nhat row 0 = den*rec = 1.0)
                    P = psP.tile([128, 384], F, tag="P")
                    for pr in range(4):
                        nc.tensor.matmul(
                            P[:], nhat[pr][:, 128 * it:128 * (it + 1)],
                            wp[:, pr, :], start=(pr == 0), stop=(pr == 3))
                    ot = sbo.tile([128, 384], H, tag="ot")
                    nc.vector.tensor_copy(ot[:], P[:])
                    nc.sync.dma_start(out=d_out[128 * it:128 * (it + 1), :],
                                      in_=ot[:])

    nc.compile()
    _CACHE["nc"] = nc
    return nc


def _host_consts(w_qkv, w_proj, b_proj):
    wqk = np.zeros((3, 128, 8, 128), np.float16)
    for k in range(3):
        rows = slice(k * 128, (k + 1) * 128)
        for pr in range(4):
            for s in range(2):  # 0 = q block, 1 = k block
                off = 384 * s
                wqk[k, :, 2 * pr + s, 0:48] = \
                    w_qkv[rows, off + 48 * (2 * pr):off + 48 * (2 * pr) + 48]
                wqk[k, :, 2 * pr + s, 64:112] = \
                    w_qkv[rows, off + 48 * (2 * pr + 1):off + 48 * (2 * pr + 1) + 48]
    wv = np.ascontiguousarray(np.transpose(
        w_qkv[:, 768:1152].reshape(3, 128, 384), (1, 0, 2))).astype(np.float16)
    wp = np.zeros((128, 4, 384), np.float16)
    for pr in range(4):  # +1: row 0 / 64 of nhat is the denominator row
        wp[1:49, pr, :] = w_proj[96 * pr:96 * pr + 48, :]
        wp[65:113, pr, :] = w_proj[96 * pr + 48:96 * pr + 96, :]
    # nhat row 0 is den*recip(den) == 1.0, so wp row 0 carries the bias
    wp[0, 0, :] = b_proj

    base = np.zeros((128, 320), np.float16)  # 1 = key allowed for query
    for r in range(4):
        for q in range(10):
            if r <= q <= r + 6:
                xj, xi = np.meshgrid(np.arange(32), np.arange(32), indexing="ij")
                base[32 * r:32 * r + 32, 32 * q:32 * q + 32] = \
                    (np.abs(xj - xi) <= 3).astype(np.float16)
    maskm = np.zeros((128, 1280), np.float16)
    for jt in range(6):
        ilo, spn, mo = SPANS[jt]
        maskm[:, JTOFF[jt]:JTOFF[jt] + spn] = base[:, mo:mo + spn]

    noobh = np.zeros((1, 512), np.float16)
    for qy in range(16):
        for qx in range(32):
            noobh[0, 32 * qy + qx] = 7.0 * (max(0, 3 - qx) + max(0, qx - 28))
    sel2 = np.zeros((1, 128), np.float16)
    sel2[0, 0] = 1.0
    sel2[0, 64] = 1.0
    bselE = np.zeros((1, 128), np.float16)
    bselE[0, 0:64] = 1.0
    bselO = np.zeros((1, 128), np.float16)
    bselO[0, 64:128] = 1.0
    d = dict(wv=wv, wp=wp, maskm=maskm, noobh=noobh, sel2=sel2,
             bselE=bselE, bselO=bselO)
    for k in range(3):
        d[f"wqk{k}"] = np.ascontiguousarray(wqk[k])
    return d


def kernel(x, w_qkv, w_proj, b_proj, H=32, W=32):
    global LAST_RESULT
    x = np.asarray(x, np.float32)
    w_qkv = np.asarray(w_qkv, np.float32)
    w_proj = np.asarray(w_proj, np.float32)
    b_proj = np.asarray(b_proj, np.float32)
    assert x.shape == (1, NPOS, C) and int(H) == 32 and int(W) == 32

    nc = _build_nc()
    consts = _host_consts(w_qkv, w_proj, b_proj)

    x4 = x[0].reshape(T, HH, WW, C)
    in_maps = []
    for c in range(8):
        t, ry0 = c // 2, 16 * (c % 2)
        xh = np.zeros((24, WW, C), np.float32)
        lo, hi = ry0 - 3, ry0 + 21
        slo, shi = max(lo, 0), min(hi, HH)
        xh[slo - lo:shi - lo] = x4[t, slo:shi]
        xTf = np.ascontiguousarray(
            xh.reshape(768, C).T.reshape(3, 128, 768)).astype(np.float16)
        m = {f"xT{k}": np.ascontiguousarray(xTf[k]) for k in range(3)}
        in_maps.append({**m, **consts})

    trace = bool(int(os.environ.get("TRACE", "0")))
    res = run_bass_kernel_spmd(nc, in_maps, core_ids=list(range(8)),
                               trace=trace)
    LAST_RESULT = res
    out = np.concatenate(
        [res.results[c]["out"].astype(np.float32) for c in range(8)], axis=0)
    return out.reshape(1, NPOS, C)


# revision 55
# speedup vs baseline: 1.1344x; 1.0674x over previous
"""Local (7x7 windowed) attention Trainium2 kernel — fp16 tensor-engine path.

Problem: B=1, N=4096 (T=4, H=W=32), C=384, 8 heads x hd=48, window 7x7
zero-padded (reference semantics: padded keys score exactly 0 -> weight
exp(0), value 0).

Sharding: data-parallel over positions. 8 cores; core c owns t-slice
c//2, query rows [16*(c%2), 16*(c%2)+16) (512 queries). Each core
recomputes k/v for a 3-row halo (24 rows = 768 halo positions,
zero-padded outside the image, matching the reference's zero padding).

v3 design: all matmul operands fp16 (1 cyc/row on the PE array vs 4 for
fp32). Per (head-pair pr, head e) the 6 key j-tiles (4 halo rows each)
pack into one [128, 1280] PSUM tile at offsets chosen so no matmul
output crosses a 2KB PSUM bank (jt2@0, jt4@320 | jt3@512, jt0@832,
jt5@960 | jt1@1024), then ONE exp ACT covers the whole tile and one DVE
multiply applies the 0/1 window mask (padded-key exp(0)=1 terms enter
the denominator via a noob-count init matmul; ones column 0 of the
augmented V produces the denominator as row 0/64 of the AV output).
All four pairs' AV outputs live in one [128, 2048] PSUM tile so the
denominator extraction, reciprocal, broadcast-to-fp16 and normalize
multiply each run once. Tiles are split per-pr / per-pt so whole-tile
dependencies pipeline (subtile dep tracking is disabled: it misses
write->read deps on strided writes here, giving nondeterministic
results).
"""

import os

# the tile scheduler's subtile dependency tracking misses write->read deps
# on this kernel's strided vaug writes (observed as nondeterministic output);
# coarse whole-tile deps are correct
os.environ["BY_DEFAULT_DISABLE_SUBTILE_DEPS"] = "1"

import numpy as np

import concourse.bacc as bacc
import concourse.mybir as mybir
import concourse.tile as tile
from concourse.bass_utils import run_bass_kernel_spmd

F = mybir.dt.float32
H = mybir.dt.float16

NH = 8
HD = 48
WIN = 7
HALF = 3
T, HH, WW = 4, 32, 32
C = 384
NPOS = T * HH * WW
SCALE = HD ** -0.5

# per j-tile (4 halo key rows each): (i_lo, span, mask_col_offset)
SPANS = [
    (0, 128, 192),
    (0, 256, 64),
    (64, 320, 0),
    (192, 320, 0),
    (320, 192, 0),
    (448, 64, 0),
]
# packed offsets of each j-tile's span inside the [128, 1280] score tile
JTOFF = {2: 0, 4: 320, 3: 512, 0: 832, 5: 960, 1: 1024}

_CACHE = {}
LAST_RESULT = None


def _build_nc():
    if "nc" in _CACHE:
        return _CACHE["nc"]
    nc = bacc.Bacc("TRN2", target_bir_lowering=False)

    d_xT = [nc.dram_tensor(f"xT{k}", [128, 768], H, kind="ExternalInput")
            for k in range(3)]
    d_wqk = [nc.dram_tensor(f"wqk{k}", [128, 8, 128], H, kind="ExternalInput")
             for k in range(3)]
    d_wv = nc.dram_tensor("wv", [128, 3, 384], H, kind="ExternalInput")
    d_wp = nc.dram_tensor("wp", [128, 4, 384], H, kind="ExternalInput")
    d_mask = nc.dram_tensor("maskm", [128, 1280], H, kind="ExternalInput")
    d_noob = nc.dram_tensor("noobh", [1, 512], H, kind="ExternalInput")
    d_sel = nc.dram_tensor("sel2", [1, 128], H, kind="ExternalInput")
    d_bselE = nc.dram_tensor("bselE", [1, 128], H, kind="ExternalInput")
    d_bselO = nc.dram_tensor("bselO", [1, 128], H, kind="ExternalInput")
    d_out = nc.dram_tensor("out", [512, 384], H, kind="ExternalOutput")

    EXP = mybir.ActivationFunctionType.Exp

    with tile.TileContext(nc) as tc:
        with tc.tile_pool(name="singles", bufs=1) as S:
            xT = [S.tile([128, 768], H, name=f"xT{k}") for k in range(3)]
            wqk = [S.tile([128, 8, 128], H, name=f"wqk{k}") for k in range(3)]
            wv = S.tile([128, 3, 384], H)
            wp = S.tile([128, 4, 384], H)
            maskm = S.tile([128, 1280], H)
            noobh = S.tile([1, 512], H)
            sel2 = S.tile([1, 128], H)
            bselE = S.tile([1, 128], H)
            bselO = S.tile([1, 128], H)
            # per-pr packed [q(512) | kA(512) | kB(256)] fp16
            qk = [S.tile([128, 1280], H, name=f"qk{pr}") for pr in range(4)]
            # per-pt augmented V: col 0 = ones (denominator), 1:49 = v
            vaug = [S.tile([128, 8, 64], H, name=f"vaug{pt}") for pt in range(6)]
            nhat = [S.tile([128, 512], H, name=f"nhat{pr}") for pr in range(4)]

            # input DMAs, ordered by first use
            for k in range(3):
                nc.sync.dma_start(out=wqk[k][:], in_=d_wqk[k][:])
                nc.sync.dma_start(out=xT[k][:], in_=d_xT[k][:])
            for sb, dr in [
                (wv, d_wv), (maskm, d_mask), (noobh, d_noob),
                (sel2, d_sel), (bselE, d_bselE), (bselO, d_bselO),
                (wp, d_wp),
            ]:
                nc.sync.dma_start(out=sb[:], in_=dr[:])
            for pt in range(6):
                nc.gpsimd.memset(vaug[pt][:, :, 0:1], 1.0)
                nc.gpsimd.memset(vaug[pt][:, :, 49:64], 0.0)

            # ---- phase 1: [q|k] per head-pair, v natural -------------
            # emit pr 0/1 first so phase-2 score matmuls can start while
            # v and the remaining pairs are still being produced
            with tc.tile_pool(name="psA", bufs=2, space="PSUM") as psA:
                def emit_qk(pr):
                    QK = psA.tile([128, 1280], F, tag="qk", name=f"QK{pr}")
                    for k in range(3):
                        st, sp_ = (k == 0), (k == 2)
                        nc.tensor.matmul(QK[:, 0:512], wqk[k][:, 2 * pr, :],
                                         xT[k][:, 96:608], start=st, stop=sp_)
                        nc.tensor.matmul(QK[:, 512:1024],
                                         wqk[k][:, 2 * pr + 1, :],
                                         xT[k][:, 0:512], start=st, stop=sp_)
                        nc.tensor.matmul(QK[:, 1024:1280],
                                         wqk[k][:, 2 * pr + 1, :],
                                         xT[k][:, 512:768], start=st, stop=sp_)
                    # split across engines to halve the copy latency
                    nc.vector.tensor_copy(qk[pr][:, 0:512], QK[:, 0:512])
                    nc.scalar.copy(qk[pr][:, 512:1280], QK[:, 512:1280])

                emit_qk(0)
                emit_qk(1)
                for pt in range(6):
                    V = psA.tile([128, 384], F, tag="V")
                    for k in range(3):
                        nc.tensor.matmul(V[:], xT[k][:, 128 * pt:128 * pt + 128],
                                         wv[:, k, :], start=(k == 0), stop=(k == 2))
                    nc.scalar.copy(
                        vaug[pt][:, :, 1:49],
                        V[:].rearrange("p (h d) -> p h d", h=8))
                emit_qk(2)
                emit_qk(3)

            # ---- phases 2-4, software-pipelined ----------------------
            # Engines execute their instruction queues in FIFO order, so
            # emission order IS schedule: interleave so no tensor-engine
            # instruction waits on a chain younger than ~2 iterations.
            with tc.tile_pool(name="psS", bufs=2, space="PSUM") as psS, \
                 tc.tile_pool(name="psO", bufs=1, space="PSUM") as psO, \
                 tc.tile_pool(name="sbe", bufs=3) as sbe, \
                 tc.tile_pool(name="sbn", bufs=2) as sbn:
                O2s = [None] * 4
                eTs = [None] * 8
                rechs = [None] * 4
                Bcs = [None] * 4

                def emit_S(i):
                    pr, e = divmod(i, 2)
                    sS = psS.tile([128, 1280], F, tag="sS", name=f"sS{i}")
                    for jt in range(6):
                        ilo, spn, _ = SPANS[jt]
                        off = JTOFF[jt]
                        nc.tensor.matmul(
                            sS[:, off:off + spn],
                            qk[pr][64 * e:64 * e + 64,
                                   512 + 128 * jt:512 + 128 * (jt + 1)],
                            qk[pr][64 * e:64 * e + 64, ilo:ilo + spn],
                            start=True, stop=True)
                    eTu = sbe.tile([128, 1280], H, tag="eTu", name=f"eTu{i}")
                    nc.scalar.activation(eTu[:], sS[:], EXP, scale=SCALE)
                    eT = sbe.tile([128, 1280], H, tag="eT", name=f"eT{i}")
                    nc.vector.tensor_mul(eT[:], eTu[:], maskm[:])
                    eTs[i] = eT

                def emit_AV(i):
                    pr, e = divmod(i, 2)
                    h = i
                    if e == 0:
                        O2s[pr] = psO.tile([128, 512], F, tag="O", bufs=2,
                                           name=f"O2_{pr}")
                        # one init for both heads: noob into den rows 0 and 64
                        nc.tensor.matmul(O2s[pr][:, :], sel2[:], noobh[:],
                                         start=True, stop=False,
                                         skip_group_check=True)
                    O2 = O2s[pr]
                    for jt in (2, 4, 3, 0, 5, 1):  # jt1 last: its eT chunk lands last
                        ilo, spn, _ = SPANS[jt]
                        off = JTOFF[jt]
                        nc.tensor.matmul(
                            O2[64 * e:64 * e + 64, ilo:ilo + spn],
                            vaug[jt][:, h, :],
                            eTs[i][:, off:off + spn],
                            start=False, stop=(jt == 1),
                            skip_group_check=True)

                def emit_chain(pr):
                    O2 = O2s[pr]
                    denB = sbn.tile([1, 512], F, tag="denB")
                    nc.scalar.copy(denB[:], O2[64:65, :])
                    recA = sbn.tile([1, 512], F, tag="recA")
                    recB = sbn.tile([1, 512], F, tag="recB")
                    # custom-DVE ops drop PSUM partition offsets; row 0 is ok
                    nc.vector.reciprocal_approx_fast(out=recA[:],
                                                     in_=O2[0:1, :])
                    nc.vector.reciprocal_approx_fast(out=recB[:], in_=denB[:])
                    rechA = sbn.tile([1, 512], H, tag="rechA")
                    rechB = sbn.tile([1, 512], H, tag="rechB")
                    nc.scalar.copy(rechA[:], recA[:])
                    nc.scalar.copy(rechB[:], recB[:])
                    rechs[pr] = (rechA, rechB)

                def emit_Bc(pr):
                    rechA, rechB = rechs[pr]
                    # reuse a psS ring slot (free once that pair's exp read it)
                    Bc = psS.tile([128, 512], F, tag="sS", name=f"Bc{pr}")
                    nc.tensor.matmul(Bc[:], bselE[:], rechA[:],
                                     start=True, stop=False)
                    nc.tensor.matmul(Bc[:], bselO[:], rechB[:],
                                     start=False, stop=True)
                    Bcs[pr] = Bc

                def emit_nhat(pr):
                    BcS = sbn.tile([128, 512], H, tag="BcS")
                    nc.scalar.copy(BcS[:], Bcs[pr][:])
                    nc.vector.tensor_mul(nhat[pr][:], O2s[pr][:], BcS[:])

                emit_S(0)
                emit_S(1)
                emit_AV(0)
                emit_S(2)
                emit_AV(1)
                emit_chain(0)
                emit_S(3)
                emit_Bc(0)
                emit_nhat(0)
                emit_AV(2)
                emit_S(4)
                emit_AV(3)
                emit_chain(1)
                emit_S(5)
                emit_Bc(1)
                emit_nhat(1)
                emit_AV(4)
                emit_S(6)
                emit_AV(5)
                emit_chain(2)
                emit_S(7)
                emit_Bc(2)
                emit_nhat(2)
                emit_AV(6)
                emit_AV(7)
                emit_chain(3)
                emit_Bc(3)
                emit_nhat(3)

            # ---- phase 5: projection + bias --------------------------
            with tc.tile_pool(name="psP", bufs=2, space="PSUM") as psP, \
                 tc.tile_pool(name="sbo", bufs=2) as sbo:
                for it in range(4):
                    # bias comes via wp row 0 (nhat row 0 = den*rec = 1.0)
                    P = psP.tile([128, 384], F, tag="P")
                    for pr in range(4):
                        nc.tensor.matmul(
                            P[:], nhat[pr][:, 128 * it:128 * (it + 1)],
                            wp[:, pr, :], start=(pr == 0), stop=(pr == 3))
                    ot = sbo.tile([128, 384], H, tag="ot")
                    nc.vector.tensor_copy(ot[:], P[:])
                    nc.sync.dma_start(out=d_out[128 * it:128 * (it + 1), :],
                                      in_=ot[:])

    nc.compile()
    _CACHE["nc"] = nc
    return nc


def _host_consts(w_qkv, w_proj, b_proj):
    wqk = np.zeros((3, 128, 8, 128), np.float16)
    for k in range(3):
        rows = slice(k * 128, (k + 1) * 128)
        for pr in range(4):
            for s in range(2):  # 0 = q block, 1 = k block
                off = 384 * s
                wqk[k, :, 2 * pr + s, 0:48] = \
                    w_qkv[rows, off + 48 * (2 * pr):off + 48 * (2 * pr) + 48]
                wqk[k, :, 2 * pr + s, 64:112] = \
                    w_qkv[rows, off + 48 * (2 * pr + 1):off + 48 * (2 * pr + 1) + 48]
    wv = np.ascontiguousarray(np.transpose(
        w_qkv[:, 768:1152].reshape(3, 128, 384), (1, 0, 2))).astype(np.float16)
    wp = np.zeros((128, 4, 384), np.float16)
    for pr in range(4):  # +1: row 0 / 64 of nhat is the denominator row
        wp[1:49, pr, :] = w_proj[96 * pr:96 * pr + 48, :]
        wp[65:113, pr, :] = w_proj[96 * pr + 48:96 * pr + 96, :]
    # nhat row 0 is den*recip(den) == 1.0, so wp row 0 carries the bias
    wp[0, 0, :] = b_proj

    base = np.zeros((128, 320), np.float16)  # 1 = key allowed for query
    for r in range(4):
        for q in range(10):
            if r <= q <= r + 6:
                xj, xi = np.meshgrid(np.arange(32), np.arange(32), indexing="ij")
                base[32 * r:32 * r + 32, 32 * q:32 * q + 32] = \
                    (np.abs(xj - xi) <= 3).astype(np.float16)
    maskm = np.zeros((128, 1280), np.float16)
    for jt in range(6):
        ilo, spn, mo = SPANS[jt]
        maskm[:, JTOFF[jt]:JTOFF[jt] + spn] = base[:, mo:mo + spn]

    noobh = np.zeros((1, 512), np.float16)
    for qy in range(16):
        for qx in range(32):
            noobh[0, 32 * qy + qx] = 7.0 * (max(0, 3 - qx) + max(0, qx - 28))
    sel2 = np.zeros((1, 128), np.float16)
    sel2[0, 0] = 1.0
    sel2[0, 64] = 1.0
    bselE = np.zeros((1, 128), np.float16)
    bselE[0, 0:64] = 1.0
    bselO = np.zeros((1, 128), np.float16)
    bselO[0, 64:128] = 1.0
    d = dict(wv=wv, wp=wp, maskm=maskm, noobh=noobh, sel2=sel2,
             bselE=bselE, bselO=bselO)
    for k in range(3):
        d[f"wqk{k}"] = np.ascontiguousarray(wqk[k])
    return d


def kernel(x, w_qkv, w_proj, b_proj, H=32, W=32):
    global LAST_RESULT
    x = np.asarray(x, np.float32)
    w_qkv = np.asarray(w_qkv, np.float32)
    w_proj = np.asarray(w_proj, np.float32)
    b_proj = np.asarray(b_proj, np.float32)
    assert x.shape == (1, NPOS, C) and int(H) == 32 and int(W) == 32

    nc = _build_nc()
    consts = _host_consts(w_qkv, w_proj, b_proj)

    x4 = x[0].reshape(T, HH, WW, C)
    in_maps = []
    for c in range(8):
        t, ry0 = c // 2, 16 * (c % 2)
        xh = np.zeros((24, WW, C), np.float32)
        lo, hi = ry0 - 3, ry0 + 21
        slo, shi = max(lo, 0), min(hi, HH)
        xh[slo - lo:shi - lo] = x4[t, slo:shi]
        xTf = np.ascontiguousarray(
            xh.reshape(768, C).T.reshape(3, 128, 768)).astype(np.float16)
        m = {f"xT{k}": np.ascontiguousarray(xTf[k]) for k in range(3)}
        in_maps.append({**m, **consts})

    trace = bool(int(os.environ.get("TRACE", "0")))
    res = run_bass_kernel_spmd(nc, in_maps, core_ids=list(range(8)),
                               trace=trace)
    LAST_RESULT = res
    out = np.concatenate(
        [res.results[c]["out"].astype(np.float32) for c in range(8)], axis=0)
    return out.reshape(1, NPOS, C)


# revision 56
# speedup vs baseline: 1.1426x; 1.0072x over previous
"""Local (7x7 windowed) attention Trainium2 kernel — fp16 tensor-engine path.

Problem: B=1, N=4096 (T=4, H=W=32), C=384, 8 heads x hd=48, window 7x7
zero-padded (reference semantics: padded keys score exactly 0 -> weight
exp(0), value 0).

Sharding: data-parallel over positions. 8 cores; core c owns t-slice
c//2, query rows [16*(c%2), 16*(c%2)+16) (512 queries). Each core
recomputes k/v for a 3-row halo (24 rows = 768 halo positions,
zero-padded outside the image, matching the reference's zero padding).

v3 design: all matmul operands fp16 (1 cyc/row on the PE array vs 4 for
fp32). Per (head-pair pr, head e) the 6 key j-tiles (4 halo rows each)
pack into one [128, 1280] PSUM tile at offsets chosen so no matmul
output crosses a 2KB PSUM bank (jt2@0, jt4@320 | jt3@512, jt0@832,
jt5@960 | jt1@1024), then ONE exp ACT covers the whole tile and one DVE
multiply applies the 0/1 window mask (padded-key exp(0)=1 terms enter
the denominator via a noob-count init matmul; ones column 0 of the
augmented V produces the denominator as row 0/64 of the AV output).
All four pairs' AV outputs live in one [128, 2048] PSUM tile so the
denominator extraction, reciprocal, broadcast-to-fp16 and normalize
multiply each run once. Tiles are split per-pr / per-pt so whole-tile
dependencies pipeline (subtile dep tracking is disabled: it misses
write->read deps on strided writes here, giving nondeterministic
results).
"""

import os

# the tile scheduler's subtile dependency tracking misses write->read deps
# on this kernel's strided vaug writes (observed as nondeterministic output);
# coarse whole-tile deps are correct
os.environ["BY_DEFAULT_DISABLE_SUBTILE_DEPS"] = "1"

import numpy as np

import concourse.bacc as bacc
import concourse.mybir as mybir
import concourse.tile as tile
from concourse.bass_utils import run_bass_kernel_spmd

F = mybir.dt.float32
H = mybir.dt.float16

NH = 8
HD = 48
WIN = 7
HALF = 3
T, HH, WW = 4, 32, 32
C = 384
NPOS = T * HH * WW
SCALE = HD ** -0.5

# per j-tile (4 halo key rows each): (i_lo, span, mask_col_offset)
SPANS = [
    (0, 128, 192),
    (0, 256, 64),
    (64, 320, 0),
    (192, 320, 0),
    (320, 192, 0),
    (448, 64, 0),
]
# packed offsets of each j-tile's span inside the [128, 1280] score tile
JTOFF = {2: 0, 4: 320, 3: 512, 0: 832, 5: 960, 1: 1024}

_CACHE = {}
LAST_RESULT = None


def _build_nc():
    if "nc" in _CACHE:
        return _CACHE["nc"]
    nc = bacc.Bacc("TRN2", target_bir_lowering=False)

    d_xT = [nc.dram_tensor(f"xT{k}", [128, 768], H, kind="ExternalInput")
            for k in range(3)]
    d_wqk = [nc.dram_tensor(f"wqk{k}", [128, 8, 128], H, kind="ExternalInput")
             for k in range(3)]
    d_wv = nc.dram_tensor("wv", [128, 3, 384], H, kind="ExternalInput")
    d_wp = nc.dram_tensor("wp", [128, 4, 384], H, kind="ExternalInput")
    d_mask = nc.dram_tensor("maskm", [128, 1280], H, kind="ExternalInput")
    d_noob = nc.dram_tensor("noobh", [1, 512], H, kind="ExternalInput")
    d_sel = nc.dram_tensor("sel2", [1, 128], H, kind="ExternalInput")
    d_bselE = nc.dram_tensor("bselE", [1, 128], H, kind="ExternalInput")
    d_bselO = nc.dram_tensor("bselO", [1, 128], H, kind="ExternalInput")
    d_out = nc.dram_tensor("out", [512, 384], H, kind="ExternalOutput")

    EXP = mybir.ActivationFunctionType.Exp

    with tile.TileContext(nc) as tc:
        with tc.tile_pool(name="singles", bufs=1) as S:
            xT = [S.tile([128, 768], H, name=f"xT{k}") for k in range(3)]
            wqk = [S.tile([128, 8, 128], H, name=f"wqk{k}") for k in range(3)]
            wv = S.tile([128, 3, 384], H)
            wp = S.tile([128, 4, 384], H)
            maskm = S.tile([128, 1280], H)
            noobh = S.tile([1, 512], H)
            sel2 = S.tile([1, 128], H)
            bselE = S.tile([1, 128], H)
            bselO = S.tile([1, 128], H)
            # per-pr packed [q(512) | kA(512) | kB(256)] fp16
            qk = [S.tile([128, 1280], H, name=f"qk{pr}") for pr in range(4)]
            # per-pt augmented V: col 0 = ones (denominator), 1:49 = v
            vaug = [S.tile([128, 8, 64], H, name=f"vaug{pt}") for pt in range(6)]
            nhat = [S.tile([128, 512], H, name=f"nhat{pr}") for pr in range(4)]

            # input DMAs, ordered by first use
            for k in range(3):
                nc.sync.dma_start(out=wqk[k][:], in_=d_wqk[k][:])
                nc.sync.dma_start(out=xT[k][:], in_=d_xT[k][:])
            for sb, dr in [
                (wv, d_wv), (maskm, d_mask), (noobh, d_noob),
                (sel2, d_sel), (bselE, d_bselE), (bselO, d_bselO),
                (wp, d_wp),
            ]:
                nc.sync.dma_start(out=sb[:], in_=dr[:])
            for pt in range(6):
                nc.gpsimd.memset(vaug[pt][:, :, 0:1], 1.0)
                nc.gpsimd.memset(vaug[pt][:, :, 49:64], 0.0)

            # ---- phase 1: [q|k] per head-pair, v natural -------------
            # emit pr 0/1 first so phase-2 score matmuls can start while
            # v and the remaining pairs are still being produced
            with tc.tile_pool(name="psA", bufs=2, space="PSUM") as psA:
                def emit_qk(pr):
                    QK = psA.tile([128, 1280], F, tag="qk", name=f"QK{pr}")
                    for k in range(3):
                        st, sp_ = (k == 0), (k == 2)
                        nc.tensor.matmul(QK[:, 0:512], wqk[k][:, 2 * pr, :],
                                         xT[k][:, 96:608], start=st, stop=sp_)
                        nc.tensor.matmul(QK[:, 512:1024],
                                         wqk[k][:, 2 * pr + 1, :],
                                         xT[k][:, 0:512], start=st, stop=sp_)
                        nc.tensor.matmul(QK[:, 1024:1280],
                                         wqk[k][:, 2 * pr + 1, :],
                                         xT[k][:, 512:768], start=st, stop=sp_)
                    # split across engines to halve the copy latency
                    nc.vector.tensor_copy(qk[pr][:, 0:512], QK[:, 0:512])
                    nc.scalar.copy(qk[pr][:, 512:1280], QK[:, 512:1280])

                emit_qk(0)
                emit_qk(1)
                for pt in range(6):
                    V = psA.tile([128, 384], F, tag="V")
                    for k in range(3):
                        nc.tensor.matmul(V[:], xT[k][:, 128 * pt:128 * pt + 128],
                                         wv[:, k, :], start=(k == 0), stop=(k == 2))
                    nc.scalar.copy(
                        vaug[pt][:, :, 1:49],
                        V[:].rearrange("p (h d) -> p h d", h=8))
                emit_qk(2)
                emit_qk(3)

            # ---- phases 2-4, software-pipelined ----------------------
            # Engines execute their instruction queues in FIFO order, so
            # emission order IS schedule: interleave so no tensor-engine
            # instruction waits on a chain younger than ~2 iterations.
            with tc.tile_pool(name="psS", bufs=2, space="PSUM") as psS, \
                 tc.tile_pool(name="psO", bufs=1, space="PSUM") as psO, \
                 tc.tile_pool(name="sbe", bufs=3) as sbe, \
                 tc.tile_pool(name="sbn", bufs=2) as sbn:
                O2s = [None] * 4
                eTs = [None] * 8
                rechs = [None] * 4
                Bcs = [None] * 4

                def emit_S(i):
                    pr, e = divmod(i, 2)
                    sS = psS.tile([128, 1280], F, tag="sS", name=f"sS{i}")
                    for jt in range(6):
                        ilo, spn, _ = SPANS[jt]
                        off = JTOFF[jt]
                        nc.tensor.matmul(
                            sS[:, off:off + spn],
                            qk[pr][64 * e:64 * e + 64,
                                   512 + 128 * jt:512 + 128 * (jt + 1)],
                            qk[pr][64 * e:64 * e + 64, ilo:ilo + spn],
                            start=True, stop=True)
                    eTu = sbe.tile([128, 1280], H, tag="eTu", name=f"eTu{i}")
                    nc.scalar.activation(eTu[:], sS[:], EXP, scale=SCALE)
                    eT = sbe.tile([128, 1280], H, tag="eT", name=f"eT{i}")
                    nc.vector.tensor_mul(eT[:], eTu[:], maskm[:])
                    eTs[i] = eT

                def emit_AV(i):
                    pr, e = divmod(i, 2)
                    h = i
                    if e == 0:
                        O2s[pr] = psO.tile([128, 512], F, tag="O", bufs=2,
                                           name=f"O2_{pr}")
                        # one init for both heads: noob into den rows 0 and 64
                        nc.tensor.matmul(O2s[pr][:, :], sel2[:], noobh[:],
                                         start=True, stop=False,
                                         skip_group_check=True)
                    O2 = O2s[pr]
                    for jt in (2, 4, 3, 0, 5, 1):  # jt1 last: its eT chunk lands last
                        ilo, spn, _ = SPANS[jt]
                        off = JTOFF[jt]
                        nc.tensor.matmul(
                            O2[64 * e:64 * e + 64, ilo:ilo + spn],
                            vaug[jt][:, h, :],
                            eTs[i][:, off:off + spn],
                            start=False, stop=(jt == 1),
                            skip_group_check=True)

                def emit_chain(pr):
                    O2 = O2s[pr]
                    denB = sbn.tile([1, 512], F, tag="denB")
                    nc.scalar.copy(denB[:], O2[64:65, :])
                    recA = sbn.tile([1, 512], F, tag="recA")
                    recB = sbn.tile([1, 512], F, tag="recB")
                    # custom-DVE ops drop PSUM partition offsets; row 0 is ok
                    nc.vector.reciprocal_approx_fast(out=recA[:],
                                                     in_=O2[0:1, :])
                    nc.vector.reciprocal_approx_fast(out=recB[:], in_=denB[:])
                    rechA = sbn.tile([1, 512], H, tag="rechA")
                    rechB = sbn.tile([1, 512], H, tag="rechB")
                    nc.scalar.copy(rechA[:], recA[:])
                    nc.scalar.copy(rechB[:], recB[:])
                    rechs[pr] = (rechA, rechB)

                def emit_Bc(pr):
                    rechA, rechB = rechs[pr]
                    # reuse a psS ring slot (free once that pair's exp read it)
                    Bc = psS.tile([128, 512], F, tag="sS", name=f"Bc{pr}")
                    nc.tensor.matmul(Bc[:], bselE[:], rechA[:],
                                     start=True, stop=False)
                    nc.tensor.matmul(Bc[:], bselO[:], rechB[:],
                                     start=False, stop=True)
                    Bcs[pr] = Bc

                def emit_nhat(pr):
                    BcS = sbn.tile([128, 512], H, tag="BcS")
                    nc.scalar.copy(BcS[:], Bcs[pr][:])
                    nc.vector.tensor_mul(nhat[pr][:], O2s[pr][:], BcS[:])

                emit_S(0)
                emit_S(1)
                emit_AV(0)
                emit_S(2)
                emit_AV(1)
                emit_chain(0)
                emit_S(3)
                emit_Bc(0)
                emit_nhat(0)
                emit_AV(2)
                emit_S(4)
                emit_AV(3)
                emit_chain(1)
                emit_S(5)
                emit_Bc(1)
                emit_nhat(1)
                emit_AV(4)
                emit_S(6)
                emit_AV(5)
                emit_chain(2)
                emit_S(7)
                emit_Bc(2)
                emit_nhat(2)
                emit_AV(6)
                emit_AV(7)
                emit_chain(3)
                emit_Bc(3)
                emit_nhat(3)

            # ---- phase 5: projection + bias --------------------------
            with tc.tile_pool(name="psP", bufs=2, space="PSUM") as psP, \
                 tc.tile_pool(name="sbo", bufs=2) as sbo:
                for it in range(4):
                    # bias comes via wp row 0 (nhat row 0 = den*rec = 1.0)
                    P = psP.tile([128, 384], F, tag="P")
                    for pr in range(4):
                        nc.tensor.matmul(
                            P[:], nhat[pr][:, 128 * it:128 * (it + 1)],
                            wp[:, pr, :], start=(pr == 0), stop=(pr == 3))
                    ot = sbo.tile([128, 384], H, tag="ot")
                    # alternate engines so the tail copies/DMAs overlap
                    if it % 2 == 0:
                        nc.vector.tensor_copy(ot[:], P[:])
                        nc.sync.dma_start(
                            out=d_out[128 * it:128 * (it + 1), :], in_=ot[:])
                    else:
                        nc.scalar.copy(ot[:], P[:])
                        nc.scalar.dma_start(
                            out=d_out[128 * it:128 * (it + 1), :], in_=ot[:])

    nc.compile()
    _CACHE["nc"] = nc
    return nc


def _host_consts(w_qkv, w_proj, b_proj):
    wqk = np.zeros((3, 128, 8, 128), np.float16)
    for k in range(3):
        rows = slice(k * 128, (k + 1) * 128)
        for pr in range(4):
            for s in range(2):  # 0 = q block, 1 = k block
                off = 384 * s
                wqk[k, :, 2 * pr + s, 0:48] = \
                    w_qkv[rows, off + 48 * (2 * pr):off + 48 * (2 * pr) + 48]
                wqk[k, :, 2 * pr + s, 64:112] = \
                    w_qkv[rows, off + 48 * (2 * pr + 1):off + 48 * (2 * pr + 1) + 48]
    wv = np.ascontiguousarray(np.transpose(
        w_qkv[:, 768:1152].reshape(3, 128, 384), (1, 0, 2))).astype(np.float16)
    wp = np.zeros((128, 4, 384), np.float16)
    for pr in range(4):  # +1: row 0 / 64 of nhat is the denominator row
        wp[1:49, pr, :] = w_proj[96 * pr:96 * pr + 48, :]
        wp[65:113, pr, :] = w_proj[96 * pr + 48:96 * pr + 96, :]
    # nhat row 0 is den*recip(den) == 1.0, so wp row 0 carries the bias
    wp[0, 0, :] = b_proj

    base = np.zeros((128, 320), np.float16)  # 1 = key allowed for query
    for r in range(4):
        for q in range(10):
            if r <= q <= r + 6:
                xj, xi = np.meshgrid(np.arange(32), np.arange(32), indexing="ij")
                base[32 * r:32 * r + 32, 32 * q:32 * q + 32] = \
                    (np.abs(xj - xi) <= 3).astype(np.float16)
    maskm = np.zeros((128, 1280), np.float16)
    for jt in range(6):
        ilo, spn, mo = SPANS[jt]
        maskm[:, JTOFF[jt]:JTOFF[jt] + spn] = base[:, mo:mo + spn]

    noobh = np.zeros((1, 512), np.float16)
    for qy in range(16):
        for qx in range(32):
            noobh[0, 32 * qy + qx] = 7.0 * (max(0, 3 - qx) + max(0, qx - 28))
    sel2 = np.zeros((1, 128), np.float16)
    sel2[0, 0] = 1.0
    sel2[0, 64] = 1.0
    bselE = np.zeros((1, 128), np.float16)
    bselE[0, 0:64] = 1.0
    bselO = np.zeros((1, 128), np.float16)
    bselO[0, 64:128] = 1.0
    d = dict(wv=wv, wp=wp, maskm=maskm, noobh=noobh, sel2=sel2,
             bselE=bselE, bselO=bselO)
    for k in range(3):
        d[f"wqk{k}"] = np.ascontiguousarray(wqk[k])
    return d


def kernel(x, w_qkv, w_proj, b_proj, H=32, W=32):
    global LAST_RESULT
    x = np.asarray(x, np.float32)
    w_qkv = np.asarray(w_qkv, np.float32)
    w_proj = np.asarray(w_proj, np.float32)
    b_proj = np.asarray(b_proj, np.float32)
    assert x.shape == (1, NPOS, C) and int(H) == 32 and int(W) == 32

    nc = _build_nc()
    consts = _host_consts(w_qkv, w_proj, b_proj)

    x4 = x[0].reshape(T, HH, WW, C)
    in_maps = []
    for c in range(8):
        t, ry0 = c // 2, 16 * (c % 2)
        xh = np.zeros((24, WW, C), np.float32)
        lo, hi = ry0 - 3, ry0 + 21
        slo, shi = max(lo, 0), min(hi, HH)
        xh[slo - lo:shi - lo] = x4[t, slo:shi]
        xTf = np.ascontiguousarray(
            xh.reshape(768, C).T.reshape(3, 128, 768)).astype(np.float16)
        m = {f"xT{k}": np.ascontiguousarray(xTf[k]) for k in range(3)}
        in_maps.append({**m, **consts})

    trace = bool(int(os.environ.get("TRACE", "0")))
    res = run_bass_kernel_spmd(nc, in_maps, core_ids=list(range(8)),
                               trace=trace)
    LAST_RESULT = res
    out = np.concatenate(
        [res.results[c]["out"].astype(np.float32) for c in range(8)], axis=0)
    return out.reshape(1, NPOS, C)


# revision 57
# speedup vs baseline: 1.1438x; 1.0010x over previous
"""Local (7x7 windowed) attention Trainium2 kernel — fp16 tensor-engine path.

Problem: B=1, N=4096 (T=4, H=W=32), C=384, 8 heads x hd=48, window 7x7
zero-padded (reference semantics: padded keys score exactly 0 -> weight
exp(0), value 0).

Sharding: data-parallel over positions. 8 cores; core c owns t-slice
c//2, query rows [16*(c%2), 16*(c%2)+16) (512 queries). Each core
recomputes k/v for a 3-row halo (24 rows = 768 halo positions,
zero-padded outside the image, matching the reference's zero padding).

v3 design: all matmul operands fp16 (1 cyc/row on the PE array vs 4 for
fp32). Per (head-pair pr, head e) the 6 key j-tiles (4 halo rows each)
pack into one [128, 1280] PSUM tile at offsets chosen so no matmul
output crosses a 2KB PSUM bank (jt2@0, jt4@320 | jt3@512, jt0@832,
jt5@960 | jt1@1024), then ONE exp ACT covers the whole tile and one DVE
multiply applies the 0/1 window mask (padded-key exp(0)=1 terms enter
the denominator via a noob-count init matmul; ones column 0 of the
augmented V produces the denominator as row 0/64 of the AV output).
All four pairs' AV outputs live in one [128, 2048] PSUM tile so the
denominator extraction, reciprocal, broadcast-to-fp16 and normalize
multiply each run once. Tiles are split per-pr / per-pt so whole-tile
dependencies pipeline (subtile dep tracking is disabled: it misses
write->read deps on strided writes here, giving nondeterministic
results).
"""

import os

# the tile scheduler's subtile dependency tracking misses write->read deps
# on this kernel's strided vaug writes (observed as nondeterministic output);
# coarse whole-tile deps are correct
os.environ["BY_DEFAULT_DISABLE_SUBTILE_DEPS"] = "1"

import numpy as np

import concourse.bacc as bacc
import concourse.mybir as mybir
import concourse.tile as tile
from concourse.bass_utils import run_bass_kernel_spmd

F = mybir.dt.float32
H = mybir.dt.float16

NH = 8
HD = 48
WIN = 7
HALF = 3
T, HH, WW = 4, 32, 32
C = 384
NPOS = T * HH * WW
SCALE = HD ** -0.5

# per j-tile (4 halo key rows each): (i_lo, span, mask_col_offset)
SPANS = [
    (0, 128, 192),
    (0, 256, 64),
    (64, 320, 0),
    (192, 320, 0),
    (320, 192, 0),
    (448, 64, 0),
]
# packed offsets of each j-tile's span inside the [128, 1280] score tile
JTOFF = {2: 0, 4: 320, 3: 512, 0: 832, 5: 960, 1: 1024}

_CACHE = {}
LAST_RESULT = None


def _build_nc():
    if "nc" in _CACHE:
        return _CACHE["nc"]
    nc = bacc.Bacc("TRN2", target_bir_lowering=False)

    d_xT = [nc.dram_tensor(f"xT{k}", [128, 768], H, kind="ExternalInput")
            for k in range(3)]
    d_wqk = [nc.dram_tensor(f"wqk{k}", [128, 8, 128], H, kind="ExternalInput")
             for k in range(3)]
    d_wv = nc.dram_tensor("wv", [128, 3, 384], H, kind="ExternalInput")
    d_wp = nc.dram_tensor("wp", [128, 4, 384], H, kind="ExternalInput")
    d_mask = nc.dram_tensor("maskm", [128, 1280], H, kind="ExternalInput")
    d_noob = nc.dram_tensor("noobh", [1, 512], H, kind="ExternalInput")
    d_sel = nc.dram_tensor("sel2", [1, 128], H, kind="ExternalInput")
    d_bselE = nc.dram_tensor("bselE", [1, 128], H, kind="ExternalInput")
    d_bselO = nc.dram_tensor("bselO", [1, 128], H, kind="ExternalInput")
    d_out = nc.dram_tensor("out", [512, 384], H, kind="ExternalOutput")

    EXP = mybir.ActivationFunctionType.Exp

    with tile.TileContext(nc) as tc:
        with tc.tile_pool(name="singles", bufs=1) as S:
            xT = [S.tile([128, 768], H, name=f"xT{k}") for k in range(3)]
            wqk = [S.tile([128, 8, 128], H, name=f"wqk{k}") for k in range(3)]
            wv = S.tile([128, 3, 384], H)
            wp = S.tile([128, 4, 384], H)
            maskm = S.tile([128, 1280], H)
            noobh = S.tile([1, 512], H)
            sel2 = S.tile([1, 128], H)
            bselE = S.tile([1, 128], H)
            bselO = S.tile([1, 128], H)
            # per-pr packed [q(512) | kA(512) | kB(256)] fp16
            qk = [S.tile([128, 1280], H, name=f"qk{pr}") for pr in range(4)]
            # per-pt augmented V: col 0 = ones (denominator), 1:49 = v
            vaug = [S.tile([128, 8, 64], H, name=f"vaug{pt}") for pt in range(6)]
            nhat = [S.tile([128, 512], H, name=f"nhat{pr}") for pr in range(4)]

            # input DMAs, ordered by first use
            for k in range(3):
                nc.sync.dma_start(out=wqk[k][:], in_=d_wqk[k][:])
                nc.sync.dma_start(out=xT[k][:], in_=d_xT[k][:])
            for sb, dr in [
                (wv, d_wv), (maskm, d_mask), (noobh, d_noob),
                (sel2, d_sel), (bselE, d_bselE), (bselO, d_bselO),
                (wp, d_wp),
            ]:
                nc.sync.dma_start(out=sb[:], in_=dr[:])
            for pt in range(6):
                nc.gpsimd.memset(vaug[pt][:, :, 0:1], 1.0)
                nc.gpsimd.memset(vaug[pt][:, :, 49:64], 0.0)

            # ---- phase 1: [q|k] per head-pair, v natural -------------
            # emit pr 0/1 first so phase-2 score matmuls can start while
            # v and the remaining pairs are still being produced
            with tc.tile_pool(name="psA", bufs=2, space="PSUM") as psA:
                def emit_qk(pr):
                    QK = psA.tile([128, 1280], F, tag="qk", name=f"QK{pr}")
                    for k in range(3):
                        st, sp_ = (k == 0), (k == 2)
                        nc.tensor.matmul(QK[:, 0:512], wqk[k][:, 2 * pr, :],
                                         xT[k][:, 96:608], start=st, stop=sp_)
                        nc.tensor.matmul(QK[:, 512:1024],
                                         wqk[k][:, 2 * pr + 1, :],
                                         xT[k][:, 0:512], start=st, stop=sp_)
                        nc.tensor.matmul(QK[:, 1024:1280],
                                         wqk[k][:, 2 * pr + 1, :],
                                         xT[k][:, 512:768], start=st, stop=sp_)
                    # split across engines to halve the copy latency
                    nc.vector.tensor_copy(qk[pr][:, 0:512], QK[:, 0:512])
                    nc.scalar.copy(qk[pr][:, 512:1280], QK[:, 512:1280])

                emit_qk(0)
                emit_qk(1)
                for pt in range(6):
                    V = psA.tile([128, 384], F, tag="V")
                    for k in range(3):
                        nc.tensor.matmul(V[:], xT[k][:, 128 * pt:128 * pt + 128],
                                         wv[:, k, :], start=(k == 0), stop=(k == 2))
                    nc.scalar.copy(
                        vaug[pt][:, :, 1:49],
                        V[:].rearrange("p (h d) -> p h d", h=8))
                emit_qk(2)
                emit_qk(3)

            # ---- phases 2-4, software-pipelined ----------------------
            # Engines execute their instruction queues in FIFO order, so
            # emission order IS schedule: interleave so no tensor-engine
            # instruction waits on a chain younger than ~2 iterations.
            with tc.tile_pool(name="psS", bufs=2, space="PSUM") as psS, \
                 tc.tile_pool(name="psO", bufs=1, space="PSUM") as psO, \
                 tc.tile_pool(name="sbe", bufs=4) as sbe, \
                 tc.tile_pool(name="sbn", bufs=3) as sbn:
                O2s = [None] * 4
                eTs = [None] * 8
                rechs = [None] * 4
                Bcs = [None] * 4

                def emit_S(i):
                    pr, e = divmod(i, 2)
                    sS = psS.tile([128, 1280], F, tag="sS", name=f"sS{i}")
                    for jt in range(6):
                        ilo, spn, _ = SPANS[jt]
                        off = JTOFF[jt]
                        nc.tensor.matmul(
                            sS[:, off:off + spn],
                            qk[pr][64 * e:64 * e + 64,
                                   512 + 128 * jt:512 + 128 * (jt + 1)],
                            qk[pr][64 * e:64 * e + 64, ilo:ilo + spn],
                            start=True, stop=True)
                    eTu = sbe.tile([128, 1280], H, tag="eTu", name=f"eTu{i}")
                    nc.scalar.activation(eTu[:], sS[:], EXP, scale=SCALE)
                    eT = sbe.tile([128, 1280], H, tag="eT", name=f"eT{i}")
                    nc.vector.tensor_mul(eT[:], eTu[:], maskm[:])
                    eTs[i] = eT

                def emit_AV(i):
                    pr, e = divmod(i, 2)
                    h = i
                    if e == 0:
                        O2s[pr] = psO.tile([128, 512], F, tag="O", bufs=2,
                                           name=f"O2_{pr}")
                        # one init for both heads: noob into den rows 0 and 64
                        nc.tensor.matmul(O2s[pr][:, :], sel2[:], noobh[:],
                                         start=True, stop=False,
                                         skip_group_check=True)
                    O2 = O2s[pr]
                    for jt in (2, 4, 3, 0, 5, 1):  # jt1 last: its eT chunk lands last
                        ilo, spn, _ = SPANS[jt]
                        off = JTOFF[jt]
                        nc.tensor.matmul(
                            O2[64 * e:64 * e + 64, ilo:ilo + spn],
                            vaug[jt][:, h, :],
                            eTs[i][:, off:off + spn],
                            start=False, stop=(jt == 1),
                            skip_group_check=True)

                def emit_chain(pr):
                    O2 = O2s[pr]
                    denB = sbn.tile([1, 512], F, tag="denB")
                    nc.scalar.copy(denB[:], O2[64:65, :])
                    recA = sbn.tile([1, 512], F, tag="recA")
                    recB = sbn.tile([1, 512], F, tag="recB")
                    # custom-DVE ops drop PSUM partition offsets; row 0 is ok
                    nc.vector.reciprocal_approx_fast(out=recA[:],
                                                     in_=O2[0:1, :])
                    nc.vector.reciprocal_approx_fast(out=recB[:], in_=denB[:])
                    rechA = sbn.tile([1, 512], H, tag="rechA")
                    rechB = sbn.tile([1, 512], H, tag="rechB")
                    nc.scalar.copy(rechA[:], recA[:])
                    nc.scalar.copy(rechB[:], recB[:])
                    rechs[pr] = (rechA, rechB)

                def emit_Bc(pr):
                    rechA, rechB = rechs[pr]
                    # reuse a psS ring slot (free once that pair's exp read it)
                    Bc = psS.tile([128, 512], F, tag="sS", name=f"Bc{pr}")
                    nc.tensor.matmul(Bc[:], bselE[:], rechA[:],
                                     start=True, stop=False)
                    nc.tensor.matmul(Bc[:], bselO[:], rechB[:],
                                     start=False, stop=True)
                    Bcs[pr] = Bc

                def emit_nhat(pr):
                    BcS = sbn.tile([128, 512], H, tag="BcS")
                    nc.scalar.copy(BcS[:], Bcs[pr][:])
                    nc.vector.tensor_mul(nhat[pr][:], O2s[pr][:], BcS[:])

                emit_S(0)
                emit_S(1)
                emit_AV(0)
                emit_S(2)
                emit_AV(1)
                emit_chain(0)
                emit_S(3)
                emit_Bc(0)
                emit_nhat(0)
                emit_AV(2)
                emit_S(4)
                emit_AV(3)
                emit_chain(1)
                emit_S(5)
                emit_Bc(1)
                emit_nhat(1)
                emit_AV(4)
                emit_S(6)
                emit_AV(5)
                emit_chain(2)
                emit_S(7)
                emit_Bc(2)
                emit_nhat(2)
                emit_AV(6)
                emit_AV(7)
                emit_chain(3)
                emit_Bc(3)
                emit_nhat(3)

            # ---- phase 5: projection + bias --------------------------
            with tc.tile_pool(name="psP", bufs=2, space="PSUM") as psP, \
                 tc.tile_pool(name="sbo", bufs=2) as sbo:
                for it in range(4):
                    # bias comes via wp row 0 (nhat row 0 = den*rec = 1.0)
                    P = psP.tile([128, 384], F, tag="P")
                    for pr in range(4):
                        nc.tensor.matmul(
                            P[:], nhat[pr][:, 128 * it:128 * (it + 1)],
                            wp[:, pr, :], start=(pr == 0), stop=(pr == 3))
                    ot = sbo.tile([128, 384], H, tag="ot")
                    # alternate engines so the tail copies/DMAs overlap
                    if it % 2 == 0:
                        nc.vector.tensor_copy(ot[:], P[:])
                        nc.sync.dma_start(
                            out=d_out[128 * it:128 * (it + 1), :], in_=ot[:])
                    else:
                        nc.scalar.copy(ot[:], P[:])
                        nc.scalar.dma_start(
                            out=d_out[128 * it:128 * (it + 1), :], in_=ot[:])

    nc.compile()
    _CACHE["nc"] = nc
    return nc


def _host_consts(w_qkv, w_proj, b_proj):
    wqk = np.zeros((3, 128, 8, 128), np.float16)
    for k in range(3):
        rows = slice(k * 128, (k + 1) * 128)
        for pr in range(4):
            for s in range(2):  # 0 = q block, 1 = k block
                off = 384 * s
                wqk[k, :, 2 * pr + s, 0:48] = \
                    w_qkv[rows, off + 48 * (2 * pr):off + 48 * (2 * pr) + 48]
                wqk[k, :, 2 * pr + s, 64:112] = \
                    w_qkv[rows, off + 48 * (2 * pr + 1):off + 48 * (2 * pr + 1) + 48]
    wv = np.ascontiguousarray(np.transpose(
        w_qkv[:, 768:1152].reshape(3, 128, 384), (1, 0, 2))).astype(np.float16)
    wp = np.zeros((128, 4, 384), np.float16)
    for pr in range(4):  # +1: row 0 / 64 of nhat is the denominator row
        wp[1:49, pr, :] = w_proj[96 * pr:96 * pr + 48, :]
        wp[65:113, pr, :] = w_proj[96 * pr + 48:96 * pr + 96, :]
    # nhat row 0 is den*recip(den) == 1.0, so wp row 0 carries the bias
    wp[0, 0, :] = b_proj

    base = np.zeros((128, 320), np.float16)  # 1 = key allowed for query
    for r in range(4):
        for q in range(10):
            if r <= q <= r + 6:
                xj, xi = np.meshgrid(np.arange(32), np.arange(32), indexing="ij")
                base[32 * r:32 * r + 32, 32 * q:32 * q + 32] = \
                    (np.abs(xj - xi) <= 3).astype(np.float16)
    maskm = np.zeros((128, 1280), np.float16)
    for jt in range(6):
        ilo, spn, mo = SPANS[jt]
        maskm[:, JTOFF[jt]:JTOFF[jt] + spn] = base[:, mo:mo + spn]

    noobh = np.zeros((1, 512), np.float16)
    for qy in range(16):
        for qx in range(32):
            noobh[0, 32 * qy + qx] = 7.0 * (max(0, 3 - qx) + max(0, qx - 28))
    sel2 = np.zeros((1, 128), np.float16)
    sel2[0, 0] = 1.0
    sel2[0, 64] = 1.0
    bselE = np.zeros((1, 128), np.float16)
    bselE[0, 0:64] = 1.0
    bselO = np.zeros((1, 128), np.float16)
    bselO[0, 64:128] = 1.0
    d = dict(wv=wv, wp=wp, maskm=maskm, noobh=noobh, sel2=sel2,
             bselE=bselE, bselO=bselO)
    for k in range(3):
        d[f"wqk{k}"] = np.ascontiguousarray(wqk[k])
    return d


def kernel(x, w_qkv, w_proj, b_proj, H=32, W=32):
    global LAST_RESULT
    x = np.asarray(x, np.float32)
    w_qkv = np.asarray(w_qkv, np.float32)
    w_proj = np.asarray(w_proj, np.float32)
    b_proj = np.asarray(b_proj, np.float32)
    assert x.shape == (1, NPOS, C) and int(H) == 32 and int(W) == 32

    nc = _build_nc()
    consts = _host_consts(w_qkv, w_proj, b_proj)

    x4 = x[0].reshape(T, HH, WW, C)
    in_maps = []
    for c in range(8):
        t, ry0 = c // 2, 16 * (c % 2)
        xh = np.zeros((24, WW, C), np.float32)
        lo, hi = ry0 - 3, ry0 + 21
        slo, shi = max(lo, 0), min(hi, HH)
        xh[slo - lo:shi - lo] = x4[t, slo:shi]
        xTf = np.ascontiguousarray(
            xh.reshape(768, C).T.reshape(3, 128, 768)).astype(np.float16)
        m = {f"xT{k}": np.ascontiguousarray(xTf[k]) for k in range(3)}
        in_maps.append({**m, **consts})

    trace = bool(int(os.environ.get("TRACE", "0")))
    res = run_bass_kernel_spmd(nc, in_maps, core_ids=list(range(8)),
                               trace=trace)
    LAST_RESULT = res
    out = np.concatenate(
        [res.results[c]["out"].astype(np.float32) for c in range(8)], axis=0)
    return out.reshape(1, NPOS, C)
